# revision 13
# baseline (speedup 1.0000x reference)
"""MoE FeedForward (top-2 of 8 experts, SwiGLU) for 8 Trainium2 NeuronCores.

Expert-parallel with top-2 sparsity: the host routes (fp32 scores,
top-2 + softmax), gathers each expert's ~N*K/E routed tokens into a
fixed-capacity buffer (C=2048), and core e computes expert e's gated
SwiGLU only for those tokens; the unshard step scatter-adds the 8
compacted partials back to token order (the MoE combine).

Layout strategy (per core):
  - x is fed pre-transposed as xT [D, N] so D (the first contraction dim)
    lies on SBUF partitions for both the router matmul and the W1/W2
    matmuls.
  - Router: computed host-side in fp32 (0.008% of the FLOPs; the
    #2-vs-#3 expert margin can be ~3e-5, inside the PE's reduced-precision
    error band, and a flipped route is a ~0.5 output error). Each core
    receives its expert's per-token gate vector g.
  - Phase B: hhT[h, tok] = silu(W1e.T @ xT) * (W2e.T @ xT), computed in
    transposed (h-on-partitions) space so no transpose is ever needed.
  - Phase C: out[tok, d] = hhT.T @ W3e with tokens back on partitions,
    so the gate multiply is a per-partition scalar on PSUM eviction.

All matmuls run as float32r (single-pass FP22) — full PE rate with
~2^-13 input precision.
"""

import numpy as np

import concourse.bacc as bacc
import concourse.bass as bass
import concourse.tile as tile
from concourse import mybir
from concourse.bass import ds, ts
from concourse.bass_utils import run_bass_kernel_spmd

AF = mybir.ActivationFunctionType
ALU = mybir.AluOpType
F32 = mybir.dt.float32
F32R = mybir.dt.float32r

# Problem shape (hardcoded per contract)
B, S, D, H, E = 2, 2048, 1024, 4096, 8
N = B * S            # 4096 tokens
TOP_K = 2
NCORES = 8

P = 128              # SBUF partitions
KD = D // P          # 8 k-tiles over D
KH = H // P          # 32 k-tiles over H
C = 1152             # per-expert token capacity (mean load is N*K/E = 1024,
                     # sigma ~28; overflow asserts loudly rather than corrupting)
NB = 384             # tokens per block (single <=512 moving chunk)
NBLK = C // NB       # 3 blocks
TT = NB // P         # 8 token-tiles per block
HT = KH              # 32 h-tiles (of 128) over H
ND = D // 512        # 2 output d-halves


def r(ap):
    """Reinterpret an f32 AP as float32r for full-rate PE matmuls."""
    return ap.bitcast(F32R)


def build_program():
    nc = bacc.Bacc(
        "TRN2",
        target_bir_lowering=False,
        debug=False,
        enable_asserts=False,
        num_devices=NCORES,
    )
    xT_d = nc.dram_tensor("xc", [D, C], F32, kind="ExternalInput").ap()
    w1_d = nc.dram_tensor("W1e", [D, H], F32, kind="ExternalInput").ap()
    w2_d = nc.dram_tensor("W2e", [D, H], F32, kind="ExternalInput").ap()
    w3_d = nc.dram_tensor("W3e", [H, D], F32, kind="ExternalInput").ap()
    g_d = nc.dram_tensor("g", [C], F32, kind="ExternalInput").ap()
    out_d = nc.dram_tensor("out", [C, D], F32, kind="ExternalOutput").ap()

    # DRAM views with the 128-partition dim innermost-of-outer
    xT_v = xT_d.bitcast(F32R).rearrange("(k p) n -> p k n", p=P)  # [128, KD, N]
    w1_v = w1_d.bitcast(F32R).rearrange("(k p) h -> p k h", p=P)  # [128, KD, H]
    w2_v = w2_d.bitcast(F32R).rearrange("(k p) h -> p k h", p=P)  # [128, KD, H]
    w3_v = w3_d.bitcast(F32R).rearrange("(k p) d -> p k d", p=P)  # [128, KH, D]
    out_v = out_d.rearrange("(t p) d -> p t d", p=P)    # [128, C/128, D]
    g_v = g_d.rearrange("(t p) -> p t", p=P)            # [128, C/128]

    with tile.TileContext(nc) as tc:
        import contextlib

        with contextlib.ExitStack() as ctx:
            singles = ctx.enter_context(tc.tile_pool(name="singles", bufs=1))
            xbp = ctx.enter_context(tc.tile_pool(name="xb", bufs=2))
            hhp = ctx.enter_context(tc.tile_pool(name="hh", bufs=2))
            wp = ctx.enter_context(tc.tile_pool(name="w", bufs=3))
            w3p = ctx.enter_context(tc.tile_pool(name="w3", bufs=3))
            evp = ctx.enter_context(tc.tile_pool(name="ev", bufs=3))
            rsp = ctx.enter_context(tc.tile_pool(name="rt", bufs=2))
            psB = ctx.enter_context(tc.tile_pool(name="psB", bufs=8, space="PSUM"))

            # Gates are computed host-side in fp32 (the router is 0.008% of
            # the FLOPs, and the #2-vs-#3 expert margin can be ~3e-5 --
            # inside the PE's reduced-precision error band, where a flipped
            # route is a ~0.5 output error).
            g_all = singles.tile([P, C // P], F32)   # per-token gate, col = token-tile
            nc.sync.dma_start(out=g_all[:], in_=g_v[:, :])

            for b in range(NBLK):
                t0 = b * NB
                # ---- load xT block: [128, KD, NB]
                xb = xbp.tile([P, KD, NB], F32R, tag="xb")
                nc.sync.dma_start(out=xb[:], in_=xT_v[:, :, ds(t0, NB)])

                # ---- Phase B: hhT[h, tok] for this block
                hh = hhp.tile([P, KH, NB], F32R, tag="hh")
                for ht in range(HT):
                    w1t = wp.tile([P, KD, P], F32R, tag="w1")
                    nc.scalar.dma_start(out=w1t[:], in_=w1_v[:, :, ts(ht, P)])
                    w2t = wp.tile([P, KD, P], F32R, tag="w2")
                    nc.scalar.dma_start(out=w2t[:], in_=w2_v[:, :, ts(ht, P)])
                    for c0 in range(0, NB, 512):
                        cw = min(512, NB - c0)
                        p1 = psB.tile([P, 512], F32, tag="ps")
                        for k in range(KD):
                            nc.tensor.matmul(
                                p1[:, :cw],
                                w1t[:, k, :],
                                xb[:, k, ds(c0, cw)],
                                start=(k == 0),
                                stop=(k == KD - 1),
                            )
                        p2 = psB.tile([P, 512], F32, tag="ps")
                        for k in range(KD):
                            nc.tensor.matmul(
                                p2[:, :cw],
                                w2t[:, k, :],
                                xb[:, k, ds(c0, cw)],
                                start=(k == 0),
                                stop=(k == KD - 1),
                            )
                        s1 = evp.tile([P, 512], F32, tag="s1")
                        nc.scalar.activation(s1[:, :cw], p1[:, :cw], AF.Silu)
                        nc.vector.tensor_mul(
                            hh[:, ht, ds(c0, cw)], s1[:, :cw], p2[:, :cw]
                        )

                # ---- Phase C: out[tok, d] = hhT.T @ W3e, gated on eviction
                for nd in range(ND):           # d-halves of 512
                    for mg in range((TT + 3) // 4):  # token-tile groups of <=4
                        gsz = min(4, TT - mg * 4)
                        pcs = []
                        for mi in range(gsz):
                            pc = psB.tile([P, 512], F32, tag="ps", name=f"pc{mi}")
                            pcs.append(pc)
                        for kh in range(KH):
                            w3t = w3p.tile([P, 512], F32R, tag="w3")
                            nc.sync.dma_start(
                                out=w3t[:], in_=w3_v[:, kh, ds(nd * 512, 512)]
                            )
                            for mi in range(gsz):
                                mt = mg * 4 + mi
                                nc.tensor.matmul(
                                    pcs[mi][:],
                                    hh[:, kh, ts(mt, P)],
                                    w3t[:],
                                    start=(kh == 0),
                                    stop=(kh == KH - 1),
                                )
                        for mi in range(gsz):
                            mt = mg * 4 + mi
                            gcol = b * TT + mt
                            ob = evp.tile([P, 512], F32, tag="ob")
                            nc.scalar.mul(ob[:], pcs[mi][:], g_all[:, gcol, None])
                            nc.sync.dma_start(
                                out=out_v[:, b * TT + mt, ds(nd * 512, 512)],
                                in_=ob[:],
                            )

    nc.compile()
    return nc


_NC_CACHE = None


def get_nc():
    global _NC_CACHE
    if _NC_CACHE is None:
        _NC_CACHE = build_program()
    return _NC_CACHE


def make_in_maps(inputs):
    x = np.asarray(inputs["x"], dtype=np.float32)
    Wg = np.ascontiguousarray(np.asarray(inputs["Wg"], dtype=np.float32))
    W1 = np.asarray(inputs["W1"], dtype=np.float32)
    W2 = np.asarray(inputs["W2"], dtype=np.float32)
    W3 = np.asarray(inputs["W3"], dtype=np.float32)

    xT = np.ascontiguousarray(x.reshape(N, D).T)        # [D, N]

    # Router on host (fp32, matches the reference's fp32 scores to ~1e-7):
    # top-2 of 8 via max / masked second-max, softmax over the selected two.
    s = x.reshape(N, D) @ Wg                            # [N, E]
    m1 = s.max(-1, keepdims=True)
    masked = np.where(s == m1, -np.inf, s)
    m2 = masked.max(-1, keepdims=True)
    den = 1.0 + np.exp(m2 - m1)
    gates = ((s >= m2) * (np.exp(s - m1) / den)).astype(np.float32)  # [N, E]

    in_maps = []
    idx_list = []
    for e in range(NCORES):
        idx = np.nonzero(gates[:, e] > 0)[0]
        assert len(idx) <= C, f"expert {e} overflow: {len(idx)} > {C}"
        idx_list.append(idx)
        xc = np.zeros((D, C), np.float32)
        xc[:, : len(idx)] = xT[:, idx]
        ge = np.zeros(C, np.float32)
        ge[: len(idx)] = gates[idx, e]
        in_maps.append(
            {
                "xc": xc,
                "W1e": np.ascontiguousarray(W1[e]),
                "W2e": np.ascontiguousarray(W2[e]),
                "W3e": np.ascontiguousarray(W3[e]),
                "g": ge,
            }
        )
    return in_maps, idx_list


def run_spmd(in_maps, trace=False, **kw):
    return run_bass_kernel_spmd(
        get_nc(), in_maps, core_ids=list(range(NCORES)), trace=trace, **kw
    )


def kernel(**inputs):
    in_maps, idx_list = make_in_maps(inputs)
    res = run_spmd(in_maps)
    out = np.zeros((N, D), np.float32)
    for e in range(NCORES):
        idx = idx_list[e]
        out[idx] += res.results[e]["out"][: len(idx)]
    return out.reshape(B, S, D)
